# revision 17
# baseline (speedup 1.0000x reference)
"""Trainium2 Bass kernel for nn_EnhancedQuantumLayer (6-qubit circuit, B=32768).

Reduction: the circuit is AngleEmbedding (per-sample RX product state) followed
by a batch-independent 64x64 unitary U (StronglyEntanglingLayers + CNOT rings,
function of `weights` only), then per-qubit PauliZ expectations.

Per sample b:
    m_b   = kron_q [cos(a_q/2), sin(a_q/2)]           (real 64-vec, a = x*scale)
    A_b   = [Re(Cc^T) ; Im(Cc^T)] m_b                 (128-vec; Cc folds the
                                                       (-i)^popcount embedding
                                                       phases into U)
    EV_bq = sum_p sgn2[p,q] * A_b[p]^2                (signs of PauliZ)

Device work per core (4096 samples): DMA x shard -> ACT sin/cos -> DVE kron
doubling (batch on partitions) -> PE pair-transposes (two 64-wide groups per
(128,128) transpose) -> packed 64->128 matmul -> ACT square -> sign matmul
(128->6) -> DMA out. Host does only the tiny weights->matrix precompute,
sharding, and the inverse column permutation.
"""
import math
from contextlib import ExitStack

import numpy as np

import concourse.bass as bass
import concourse.mybir as mybir
from concourse.bass_utils import run_bass_kernel_spmd

F32 = mybir.dt.float32
NQ = 6
NL = 6
B = 32768
NCORES = 8
BC = B // NCORES          # 4096 samples per core
NGROUPS = BC // 128       # 32 groups of 128 samples
NSUPER = 4                # super-chunks of 8 groups (1024 samples)


# ---------------------------------------------------------------- host precompute
def _host_matrices(weights):
    """(CcPacked (64,128) f32, SgnZ2 (128,6) f32) from weights (6,6,3)."""
    w = np.asarray(weights, dtype=np.float64)
    phi, theta, omega = w[..., 0], w[..., 1], w[..., 2]
    ct, st = np.cos(0.5 * theta), np.sin(0.5 * theta)
    em = np.exp(-0.5j * (phi + omega))
    ep = np.exp(0.5j * (phi + omega))
    epm = np.exp(0.5j * (phi - omega))
    emp = np.exp(-0.5j * (phi - omega))

    state = np.eye(64, dtype=np.complex128).reshape((64,) + (2,) * NQ)

    def apply_1q(state, U, q):
        ax = q + 1
        s = np.moveaxis(state, ax, -1)
        s = np.einsum('ij,...j->...i', U, s)
        return np.moveaxis(s, -1, ax)

    def cnot(state, c, t):
        ca, ta = c + 1, t + 1
        s0 = np.take(state, 0, axis=ca)
        s1 = np.take(state, 1, axis=ca)
        t_in = ta - 1 if ta > ca else ta
        s1 = np.flip(s1, axis=t_in)
        return np.stack([s0, s1], axis=ca)

    for l in range(NL):
        for q in range(NQ):
            U = np.array([
                [em[l, q] * ct[l, q], -epm[l, q] * st[l, q]],
                [emp[l, q] * st[l, q], ep[l, q] * ct[l, q]],
            ])
            state = apply_1q(state, U, q)
        r = (l % (NQ - 1)) + 1
        for q in range(NQ):
            state = cnot(state, q, (q + r) % NQ)

    stateF = state.reshape(64, 64)            # [in_e, out_o] = U[o, e]
    e = np.arange(64)
    pc = np.array([bin(v).count('1') for v in e])
    phase = (-1j) ** pc                       # (-i)^popcount: RX embedding phases
    Cc = phase[:, None] * stateF              # (64_in, 64_out)

    # device row j has qubit q at bit q; reference index e has qubit 0 as MSB
    bitrev = np.array([int(format(j, '06b')[::-1], 2) for j in range(64)])
    Cdev = Cc[bitrev, :]

    ccpacked = np.concatenate([Cdev.real, Cdev.imag], axis=1)   # (64, 128)

    o = np.arange(64)
    z = np.stack([1.0 - 2.0 * ((o >> (5 - q)) & 1) for q in range(NQ)], axis=1)
    sgn2 = np.concatenate([z, z], axis=0)                        # (128, 6)
    return ccpacked.astype(np.float32), sgn2.astype(np.float32)


def _out_perm():
    """col g in device out (6, BC) holds sample_local perm[g]."""
    perm = np.empty(BC, np.int64)
    p = np.arange(128)
    for ch in range(2 * NSUPER):
        s, h = divmod(ch, 2)
        for tt in range(4):
            n = 8 * s + 2 * tt + h
            perm[ch * 512 + tt * 128 + p] = 32 * p + n
    return perm


# ---------------------------------------------------------------- device program
def _build_bass(reps=1):
    """Raw-bass pipeline with hand-placed semaphores.

    Engine programs (per core, 4096 samples, per iteration):
      SP    in-DMA -> (wait ACT done) out-DMA
      ACT   sin, cos, then per chunk k: square(k), ev-copy(k)   (18 ticks)
      DVE   kron doubling (12 ops), 4x M_T copy                 (16 ticks)
      PE    16 pair-transposes, 8 packed matmuls, 8 sign matmuls (32 ticks)

    Walrus allows very few semaphore waits per compute instruction, so all
    waits are standalone sequencer wait_ge ops. reps>1 re-runs the body
    end-to-end (serialized on the xt buffer) for differential timing.
    """
    nc = bass.Bass()
    # xin columns: [x data (192) | half_scale | pi/2 | Cc packed twice (128)
    #               | sgn2 (6) | identity (128)]
    xin = nc.dram_tensor("xin", [128, 456], F32, kind="ExternalInput")
    out = nc.dram_tensor("out", [NQ, BC], F32, kind="ExternalOutput")

    # per-iteration tick totals
    AT, VT, PT, DT = 18, 16, 32, 16
    # PE tick tables (match emission order below)
    main_tick = [9, 14, 20, 22, 24, 26, 28, 30]
    sign_tick = [15, 21, 23, 25, 27, 29, 31, 32]
    lastT_tick = [4, 8, 13, 19]
    sq_tick = [3, 4, 6, 8, 10, 12, 14, 16]

    ctx = ExitStack()
    with ctx:
        sb = lambda nm, shape: ctx.enter_context(nc.sbuf_tensor(nm, shape, F32))
        ps = lambda nm, shape: ctx.enter_context(nc.psum_tensor(nm, shape, F32))
        sem = lambda nm: ctx.enter_context(nc.semaphore(name=nm))

        xt = sb("xt", [128, 456])
        snt = sb("snt", [128, 192])
        cst = sb("cst", [128, 192])
        ma = sb("ma", [128, 2048])
        mb = sb("mb", [128, 2048])
        mts = [sb(f"mts{i}", [128, 512]) for i in range(4)]
        pps = [sb(f"pps{i}", [128, 512]) for i in range(8)]
        ev_sb = sb("ev_sb", [NQ, BC])
        pts = [ps(f"pts{i}", [128, 512]) for i in range(2)]
        ams = [ps(f"ams{i}", [128, 512]) for i in range(2)]
        evs = [ps(f"evs{i}", [NQ, 512]) for i in range(2)]

        Sd, Sa, Sv, Sp, So = (sem("Sd"), sem("Sa"), sem("Sv"), sem("Sp"),
                              sem("So"))

        cc2 = xt.ap()[:, 194:322]
        sg_t = xt.ap()[:, 322:328]
        ident = xt.ap()[:, 328:456]
        mfin = [ma, mb][(NQ - 1) % 2]

        block = ctx.enter_context(nc.Block())

        @block.sync
        def _(sync):
            for i in range(reps):
                if i > 0:
                    sync.wait_ge(Sp, PT * i)      # xt free (sign7 of i-1)
                sync.dma_start(out=xt.ap()[:, :], in_=xin[:, :]).then_inc(Sd, 16)
                sync.wait_ge(Sa, AT * (i + 1))
                sync.dma_start(
                    out=out[:, :], in_=ev_sb.ap()[:NQ, :]
                ).then_inc(So, 16)
            sync.wait_ge(So, 16 * reps)

        @block.scalar
        def _(scalar):
            sfn = mybir.ActivationFunctionType.Sin
            for i in range(reps):
                ab, vb, pb = AT * i, VT * i, PT * i
                scalar.wait_ge(Sd, 16 * (i + 1))
                if i > 0:
                    scalar.wait_ge(Sv, vb - 4)    # kron(i-1) done with snt/cst
                nc.scalar.activation(
                    snt.ap()[:, :], xt.ap()[:, 0:192], sfn,
                    scale=xt.ap()[:, 192:193],
                ).then_inc(Sa, 1)
                nc.scalar.activation(
                    cst.ap()[:, :], xt.ap()[:, 0:192], sfn,
                    bias=xt.ap()[:, 193:194], scale=xt.ap()[:, 192:193],
                ).then_inc(Sa, 1)
                for k in range(8):
                    scalar.wait_ge(Sp, pb + main_tick[k])
                    nc.scalar.square(
                        pps[k].ap()[:, :], ams[k % 2].ap()[:, :]
                    ).then_inc(Sa, 1)
                    if k == 1:
                        scalar.wait_ge(So, 16 * i)  # out-DMA(i-1) done reading
                    if k >= 1:
                        scalar.wait_ge(Sp, pb + sign_tick[k - 1])
                        nc.scalar.copy(
                            ev_sb.ap()[:NQ, (k - 1) * 512:k * 512],
                            evs[(k - 1) % 2].ap()[:, :],
                        ).then_inc(Sa, 1)
                scalar.wait_ge(Sp, pb + sign_tick[7])
                nc.scalar.copy(
                    ev_sb.ap()[:NQ, 7 * 512:8 * 512], evs[7 % 2].ap()[:, :]
                ).then_inc(Sa, 1)

        @block.vector
        def _(vector):
            cs3 = cst.ap()[:, :].rearrange("p (g c) -> p g c", g=NGROUPS)
            sn3 = snt.ap()[:, :].rearrange("p (g c) -> p g c", g=NGROUPS)
            for i in range(reps):
                ab, vb, pb = AT * i, VT * i, PT * i
                vector.wait_ge(Sa, ab + 2)
                if i > 0:
                    vector.wait_ge(Sp, pb - PT + 19)  # transposes(i-1) done
                bufs = [ma, mb]
                cur = bufs[0].ap()[:, :].rearrange("p (g w) -> p g w", g=NGROUPS)
                nc.vector.tensor_copy(cur[:, :, 0:1], cs3[:, :, 0:1]).then_inc(Sv, 1)
                nc.vector.tensor_copy(cur[:, :, 1:2], sn3[:, :, 0:1]).then_inc(Sv, 1)
                for q in range(1, NQ):
                    w = 1 << q
                    nxt = bufs[q % 2].ap()[:, :].rearrange(
                        "p (g w) -> p g w", g=NGROUPS
                    )
                    cb = cs3[:, :, q:q + 1].broadcast_to((128, NGROUPS, w))
                    sb_ = sn3[:, :, q:q + 1].broadcast_to((128, NGROUPS, w))
                    vector.wait_ge(Sv, vb + 2 * q)  # deep-pipeline RAW
                    nc.vector.tensor_mul(
                        nxt[:, :, 0:w], cur[:, :, 0:w], cb
                    ).then_inc(Sv, 1)
                    nc.vector.tensor_mul(
                        nxt[:, :, w:2 * w], cur[:, :, 0:w], sb_
                    ).then_inc(Sv, 1)
                    cur = nxt
                for s in range(4):
                    vector.wait_ge(Sp, pb + lastT_tick[s])
                    nc.vector.tensor_copy(
                        mts[s].ap()[:, :], pts[s % 2].ap()[:, :]
                    ).then_inc(Sv, 1)

        @block.tensor
        def _(tensor):
            def transpose_group(g0):
                for t in range(4):
                    g = g0 + t
                    nc.tensor.transpose(
                        pts[(g // 4) % 2].ap()[:, t * 128:(t + 1) * 128],
                        mfin.ap()[:, g * 128:(g + 1) * 128],
                        ident,
                    ).then_inc(Sp, 1)

            def main(k):
                s, h = divmod(k, 2)
                nc.tensor.matmul(
                    ams[k % 2].ap()[:, :],
                    cc2[64 * h:64 * h + 64, :],
                    mts[s].ap()[64 * h:64 * h + 64, :],
                    start=True, stop=True,
                ).then_inc(Sp, 1)

            def sign(k):
                nc.tensor.matmul(
                    evs[k % 2].ap()[:, :], sg_t, pps[k].ap()[:, :],
                    start=True, stop=True,
                ).then_inc(Sp, 1)

            for i in range(reps):
                ab, vb, pb = AT * i, VT * i, PT * i
                tensor.wait_ge(Sv, vb + 12)
                transpose_group(0)                       # ticks 1-4   pt0
                transpose_group(4)                       # ticks 5-8   pt1
                tensor.wait_ge(Sv, vb + 13)
                main(0)                                  # tick 9
                transpose_group(8)                       # ticks 10-13 pt0
                main(1)                                  # tick 14
                tensor.wait_ge(Sa, ab + sq_tick[0])
                sign(0)                                  # tick 15
                tensor.wait_ge(Sv, vb + 14)
                transpose_group(12)                      # ticks 16-19 pt1
                main(2)                                  # tick 20
                tensor.wait_ge(Sa, ab + sq_tick[1])
                sign(1)                                  # tick 21
                main(3)                                  # tick 22
                tensor.wait_ge(Sa, ab + sq_tick[2])
                sign(2)                                  # tick 23
                tensor.wait_ge(Sv, vb + 15)
                main(4)                                  # tick 24
                tensor.wait_ge(Sa, ab + sq_tick[3])
                sign(3)                                  # tick 25
                main(5)                                  # tick 26
                tensor.wait_ge(Sa, ab + sq_tick[4])
                sign(4)                                  # tick 27
                tensor.wait_ge(Sv, vb + 16)
                main(6)                                  # tick 28
                tensor.wait_ge(Sa, ab + sq_tick[5])
                sign(5)                                  # tick 29
                main(7)                                  # tick 30
                tensor.wait_ge(Sa, ab + sq_tick[6])
                sign(6)                                  # tick 31
                tensor.wait_ge(Sa, ab + sq_tick[7])
                sign(7)                                  # tick 32

    return nc


_CACHE = {}


def _get_nc():
    if "nc" not in _CACHE:
        _CACHE["nc"] = _build_bass()
        _CACHE["perm"] = _out_perm()
    return _CACHE["nc"], _CACHE["perm"]


# ---------------------------------------------------------------- entry point
def _make_in_maps(x, weights, scale):
    x = np.ascontiguousarray(np.asarray(x, dtype=np.float32))
    ccp, sg2 = _host_matrices(weights)
    hscale = 0.5 * float(np.asarray(scale).reshape(-1)[0])
    in_maps = []
    for k in range(NCORES):
        xs2 = np.empty((128, 456), np.float32)
        xs2[:, 0:192] = x[k * BC:(k + 1) * BC].reshape(128, 192)
        xs2[:, 192] = hscale
        xs2[:, 193] = math.pi / 2
        xs2[0:64, 194:322] = ccp
        xs2[64:128, 194:322] = ccp
        xs2[:, 322:328] = sg2
        xs2[:, 328:456] = np.eye(128, dtype=np.float32)
        in_maps.append({"xin": xs2})
    return in_maps


def kernel(x, weights, scale):
    nc, perm = _get_nc()
    in_maps = _make_in_maps(x, weights, scale)
    res = run_bass_kernel_spmd(nc, in_maps, list(range(NCORES))).results
    ev = np.empty((B, NQ), np.float32)
    for k in range(NCORES):
        ev[k * BC + perm, :] = res[k]["out"].T
    return ev


if __name__ == "__main__":
    rng = np.random.default_rng(0)
    x = rng.standard_normal((B, NQ)).astype(np.float32)
    weights = rng.uniform(0, 2 * np.pi, (NL, NQ, 3)).astype(np.float32)
    scale = np.array([0.1], np.float32)
    ev = kernel(x, weights, scale)
    print("out", ev.shape, ev.dtype, ev[:2])


# revision 18
# speedup vs baseline: 2.2573x; 2.2573x over previous
"""Trainium2 Bass kernel for nn_EnhancedQuantumLayer (6-qubit circuit, B=32768).

Reduction: the circuit is AngleEmbedding (per-sample RX product state) followed
by a batch-independent 64x64 unitary U (StronglyEntanglingLayers + CNOT rings,
function of `weights` only), then per-qubit PauliZ expectations.

Per sample b:
    m_b   = kron_q [cos(a_q/2), sin(a_q/2)]           (real 64-vec, a = x*scale)
    A_b   = [Re(Cc^T) ; Im(Cc^T)] m_b                 (128-vec; Cc folds the
                                                       (-i)^popcount embedding
                                                       phases into U)
    EV_bq = sum_p sgn2[p,q] * A_b[p]^2                (signs of PauliZ)

Device work per core (4096 samples): DMA x shard -> ACT sin/cos -> DVE kron
doubling (batch on partitions) -> PE pair-transposes (two 64-wide groups per
(128,128) transpose) -> packed 64->128 matmul -> ACT square -> sign matmul
(128->6) -> DMA out. Host does only the tiny weights->matrix precompute,
sharding, and the inverse column permutation.
"""
import math
from contextlib import ExitStack

import numpy as np

import concourse.bass as bass
import concourse.mybir as mybir
from concourse.bass_utils import run_bass_kernel_spmd

F32 = mybir.dt.float32
NQ = 6
NL = 6
B = 32768
NCORES = 8
BC = B // NCORES          # 4096 samples per core
NGROUPS = BC // 128       # 32 groups of 128 samples
NSUPER = 4                # super-chunks of 8 groups (1024 samples)


# ---------------------------------------------------------------- host precompute
def _host_matrices(weights):
    """(CcPacked (64,128) f32, SgnZ2 (128,6) f32) from weights (6,6,3)."""
    w = np.asarray(weights, dtype=np.float64)
    phi, theta, omega = w[..., 0], w[..., 1], w[..., 2]
    ct, st = np.cos(0.5 * theta), np.sin(0.5 * theta)
    em = np.exp(-0.5j * (phi + omega))
    ep = np.exp(0.5j * (phi + omega))
    epm = np.exp(0.5j * (phi - omega))
    emp = np.exp(-0.5j * (phi - omega))

    state = np.eye(64, dtype=np.complex128).reshape((64,) + (2,) * NQ)

    def apply_1q(state, U, q):
        ax = q + 1
        s = np.moveaxis(state, ax, -1)
        s = np.einsum('ij,...j->...i', U, s)
        return np.moveaxis(s, -1, ax)

    def cnot(state, c, t):
        ca, ta = c + 1, t + 1
        s0 = np.take(state, 0, axis=ca)
        s1 = np.take(state, 1, axis=ca)
        t_in = ta - 1 if ta > ca else ta
        s1 = np.flip(s1, axis=t_in)
        return np.stack([s0, s1], axis=ca)

    for l in range(NL):
        for q in range(NQ):
            U = np.array([
                [em[l, q] * ct[l, q], -epm[l, q] * st[l, q]],
                [emp[l, q] * st[l, q], ep[l, q] * ct[l, q]],
            ])
            state = apply_1q(state, U, q)
        r = (l % (NQ - 1)) + 1
        for q in range(NQ):
            state = cnot(state, q, (q + r) % NQ)

    stateF = state.reshape(64, 64)            # [in_e, out_o] = U[o, e]
    e = np.arange(64)
    pc = np.array([bin(v).count('1') for v in e])
    phase = (-1j) ** pc                       # (-i)^popcount: RX embedding phases
    Cc = phase[:, None] * stateF              # (64_in, 64_out)

    # device row j has qubit q at bit q; reference index e has qubit 0 as MSB
    bitrev = np.array([int(format(j, '06b')[::-1], 2) for j in range(64)])
    Cdev = Cc[bitrev, :]

    ccpacked = np.concatenate([Cdev.real, Cdev.imag], axis=1)   # (64, 128)

    o = np.arange(64)
    z = np.stack([1.0 - 2.0 * ((o >> (5 - q)) & 1) for q in range(NQ)], axis=1)
    sgn2 = np.concatenate([z, z], axis=0)                        # (128, 6)
    return ccpacked.astype(np.float32), sgn2.astype(np.float32)


def _out_perm():
    """col g in device out (6, BC) holds sample_local perm[g]."""
    perm = np.empty(BC, np.int64)
    p = np.arange(128)
    for ch in range(2 * NSUPER):
        s, h = divmod(ch, 2)
        for tt in range(4):
            n = 8 * s + 2 * tt + h
            perm[ch * 512 + tt * 128 + p] = 32 * p + n
    return perm


# ---------------------------------------------------------------- device program
def _build_bass(reps=1):
    """Raw-bass pipeline with hand-placed semaphores.

    Engine programs (per core, 4096 samples, per iteration):
      SP    in-DMA -> (wait ACT done) out-DMA
      ACT   sin, cos, then per chunk k: square(k), ev-copy(k)   (18 ticks)
      DVE   kron doubling (12 ops), 4x M_T copy                 (16 ticks)
      PE    16 pair-transposes, 8 packed matmuls, 8 sign matmuls (32 ticks)

    Walrus allows very few semaphore waits per compute instruction, so all
    waits are standalone sequencer wait_ge ops. reps>1 re-runs the body
    end-to-end (serialized on the xt buffer) for differential timing.
    """
    nc = bass.Bass()
    # xin columns: [x data (192) | half_scale | pi/2 | Cc packed twice (128)
    #               | sgn2 (6) | identity (128)]
    xin = nc.dram_tensor("xin", [128, 456], F32, kind="ExternalInput")
    out = nc.dram_tensor("out", [NQ, BC], F32, kind="ExternalOutput")

    # per-iteration tick totals
    AT, VT, PT, DT = 18, 16, 32, 16
    # PE tick tables (match emission order below)
    main_tick = [9, 14, 20, 22, 24, 26, 28, 30]
    sign_tick = [15, 21, 23, 25, 27, 29, 31, 32]
    lastT_tick = [4, 8, 13, 19]
    sq_tick = [3, 4, 6, 8, 10, 12, 14, 16]

    ctx = ExitStack()
    with ctx:
        sb = lambda nm, shape: ctx.enter_context(nc.sbuf_tensor(nm, shape, F32))
        ps = lambda nm, shape: ctx.enter_context(nc.psum_tensor(nm, shape, F32))
        sem = lambda nm: ctx.enter_context(nc.semaphore(name=nm))

        xt = sb("xt", [128, 456])
        snt = sb("snt", [128, 192])
        cst = sb("cst", [128, 192])
        ma = sb("ma", [128, 2048])
        mb = sb("mb", [128, 2048])
        mts = [sb(f"mts{i}", [128, 512]) for i in range(4)]
        pps = [sb(f"pps{i}", [128, 512]) for i in range(8)]
        ev_sb = sb("ev_sb", [NQ, BC])
        pts = [ps(f"pts{i}", [128, 512]) for i in range(2)]
        ams = [ps(f"ams{i}", [128, 512]) for i in range(2)]
        evs = [ps(f"evs{i}", [NQ, 512]) for i in range(2)]

        Sd, Sa, Sv, Sp, So = (sem("Sd"), sem("Sa"), sem("Sv"), sem("Sp"),
                              sem("So"))

        cc2 = xt.ap()[:, 194:322]
        sg_t = xt.ap()[:, 322:328]
        ident = xt.ap()[:, 328:456]
        mfin = [ma, mb][(NQ - 1) % 2]

        block = ctx.enter_context(nc.Block())

        def w(inst, sem, val):
            return inst._wait_ge(sem, val)

        @block.sync
        def _(sync):
            for i in range(reps):
                d = sync.dma_start(out=xt.ap()[:, :], in_=xin[:, :])
                if i > 0:
                    w(d, Sp, PT * i)              # xt free (sign7 of i-1)
                d.then_inc(Sd, 16)
                o = sync.dma_start(out=out[:, :], in_=ev_sb.ap()[:NQ, :])
                w(o, Sa, AT * (i + 1)).then_inc(So, 16)
            sync.wait_ge(So, 16 * reps)

        @block.scalar
        def _(scalar):
            sfn = mybir.ActivationFunctionType.Sin
            for i in range(reps):
                ab, pb = AT * i, PT * i
                s_ = nc.scalar.activation(
                    snt.ap()[:, :], xt.ap()[:, 0:192], sfn,
                    scale=xt.ap()[:, 192:193],
                )
                w(s_, Sd, 16 * (i + 1)).then_inc(Sa, 1)
                c_ = nc.scalar.activation(
                    cst.ap()[:, :], xt.ap()[:, 0:192], sfn,
                    bias=xt.ap()[:, 193:194], scale=xt.ap()[:, 192:193],
                )
                if i > 0:
                    w(c_, So, 16 * i)             # ev_sb free (out-DMA i-1)
                c_.then_inc(Sa, 1)
                for k in range(8):
                    q_ = nc.scalar.square(pps[k].ap()[:, :], ams[k % 2].ap()[:, :])
                    w(q_, Sp, pb + main_tick[k]).then_inc(Sa, 1)
                    if k >= 1:
                        e_ = nc.scalar.copy(
                            ev_sb.ap()[:NQ, (k - 1) * 512:k * 512],
                            evs[(k - 1) % 2].ap()[:, :],
                        )
                        w(e_, Sp, pb + sign_tick[k - 1]).then_inc(Sa, 1)
                e_ = nc.scalar.copy(
                    ev_sb.ap()[:NQ, 7 * 512:8 * 512], evs[7 % 2].ap()[:, :]
                )
                w(e_, Sp, pb + sign_tick[7]).then_inc(Sa, 1)

        @block.vector
        def _(vector):
            cs3 = cst.ap()[:, :].rearrange("p (g c) -> p g c", g=NGROUPS)
            sn3 = snt.ap()[:, :].rearrange("p (g c) -> p g c", g=NGROUPS)
            for i in range(reps):
                ab, vb, pb = AT * i, VT * i, PT * i
                bufs = [ma, mb]
                cur = bufs[0].ap()[:, :].rearrange("p (g w) -> p g w", g=NGROUPS)
                c1 = nc.vector.tensor_copy(cur[:, :, 0:1], cs3[:, :, 0:1])
                w(c1, Sa, ab + 2).then_inc(Sv, 1)
                c2 = nc.vector.tensor_copy(cur[:, :, 1:2], sn3[:, :, 0:1])
                if i > 0:
                    w(c2, Sp, pb - PT + 19)       # mb free (transposes i-1)
                c2.then_inc(Sv, 1)
                for q in range(1, NQ):
                    ww = 1 << q
                    nxt = bufs[q % 2].ap()[:, :].rearrange(
                        "p (g w) -> p g w", g=NGROUPS
                    )
                    cb = cs3[:, :, q:q + 1].broadcast_to((128, NGROUPS, ww))
                    sb_ = sn3[:, :, q:q + 1].broadcast_to((128, NGROUPS, ww))
                    m1 = nc.vector.tensor_mul(nxt[:, :, 0:ww], cur[:, :, 0:ww], cb)
                    w(m1, Sv, vb + 2 * q).then_inc(Sv, 1)   # deep-pipeline RAW
                    nc.vector.tensor_mul(
                        nxt[:, :, ww:2 * ww], cur[:, :, 0:ww], sb_
                    ).then_inc(Sv, 1)
                    cur = nxt
                for s in range(4):
                    mc = nc.vector.tensor_copy(mts[s].ap()[:, :], pts[s % 2].ap()[:, :])
                    w(mc, Sp, pb + lastT_tick[s]).then_inc(Sv, 1)

        @block.tensor
        def _(tensor):
            def transpose_group(g0, wait=None):
                for t in range(4):
                    g = g0 + t
                    tr = nc.tensor.transpose(
                        pts[(g // 4) % 2].ap()[:, t * 128:(t + 1) * 128],
                        mfin.ap()[:, g * 128:(g + 1) * 128],
                        ident,
                    )
                    if t == 0 and wait is not None:
                        w(tr, *wait)
                    tr.then_inc(Sp, 1)

            def main(k, wait=None):
                s, h = divmod(k, 2)
                mm = nc.tensor.matmul(
                    ams[k % 2].ap()[:, :],
                    cc2[64 * h:64 * h + 64, :],
                    mts[s].ap()[64 * h:64 * h + 64, :],
                    start=True, stop=True,
                )
                if wait is not None:
                    w(mm, *wait)
                mm.then_inc(Sp, 1)

            def sign(k, wait=None):
                mm = nc.tensor.matmul(
                    evs[k % 2].ap()[:, :], sg_t, pps[k].ap()[:, :],
                    start=True, stop=True,
                )
                if wait is not None:
                    w(mm, *wait)
                mm.then_inc(Sp, 1)

            for i in range(reps):
                ab, vb = AT * i, VT * i
                transpose_group(0, wait=(Sv, vb + 12))   # ticks 1-4   pt0
                transpose_group(4)                       # ticks 5-8   pt1
                main(0, wait=(Sv, vb + 13))              # tick 9
                transpose_group(8)                       # ticks 10-13 pt0
                main(1)                                  # tick 14
                sign(0, wait=(Sa, ab + sq_tick[0]))      # tick 15
                transpose_group(12, wait=(Sv, vb + 14))  # ticks 16-19 pt1
                main(2)                                  # tick 20
                sign(1, wait=(Sa, ab + sq_tick[1]))      # tick 21
                main(3)                                  # tick 22
                sign(2, wait=(Sa, ab + sq_tick[2]))      # tick 23
                main(4, wait=(Sv, vb + 15))              # tick 24
                sign(3, wait=(Sa, ab + sq_tick[3]))      # tick 25
                main(5)                                  # tick 26
                sign(4, wait=(Sa, ab + sq_tick[4]))      # tick 27
                main(6, wait=(Sv, vb + 16))              # tick 28
                sign(5, wait=(Sa, ab + sq_tick[5]))      # tick 29
                main(7)                                  # tick 30
                sign(6, wait=(Sa, ab + sq_tick[6]))      # tick 31
                sign(7, wait=(Sa, ab + sq_tick[7]))      # tick 32

    return nc


_CACHE = {}


def _get_nc():
    if "nc" not in _CACHE:
        _CACHE["nc"] = _build_bass()
        _CACHE["perm"] = _out_perm()
    return _CACHE["nc"], _CACHE["perm"]


# ---------------------------------------------------------------- entry point
def _make_in_maps(x, weights, scale):
    x = np.ascontiguousarray(np.asarray(x, dtype=np.float32))
    ccp, sg2 = _host_matrices(weights)
    hscale = 0.5 * float(np.asarray(scale).reshape(-1)[0])
    in_maps = []
    for k in range(NCORES):
        xs2 = np.empty((128, 456), np.float32)
        xs2[:, 0:192] = x[k * BC:(k + 1) * BC].reshape(128, 192)
        xs2[:, 192] = hscale
        xs2[:, 193] = math.pi / 2
        xs2[0:64, 194:322] = ccp
        xs2[64:128, 194:322] = ccp
        xs2[:, 322:328] = sg2
        xs2[:, 328:456] = np.eye(128, dtype=np.float32)
        in_maps.append({"xin": xs2})
    return in_maps


def kernel(x, weights, scale):
    nc, perm = _get_nc()
    in_maps = _make_in_maps(x, weights, scale)
    res = run_bass_kernel_spmd(nc, in_maps, list(range(NCORES))).results
    ev = np.empty((B, NQ), np.float32)
    for k in range(NCORES):
        ev[k * BC + perm, :] = res[k]["out"].T
    return ev


if __name__ == "__main__":
    rng = np.random.default_rng(0)
    x = rng.standard_normal((B, NQ)).astype(np.float32)
    weights = rng.uniform(0, 2 * np.pi, (NL, NQ, 3)).astype(np.float32)
    scale = np.array([0.1], np.float32)
    ev = kernel(x, weights, scale)
    print("out", ev.shape, ev.dtype, ev[:2])


# revision 19
# speedup vs baseline: 2.4836x; 1.1003x over previous
"""Trainium2 Bass kernel for nn_EnhancedQuantumLayer (6-qubit circuit, B=32768).

Reduction: the circuit is AngleEmbedding (per-sample RX product state) followed
by a batch-independent 64x64 unitary U (StronglyEntanglingLayers + CNOT rings,
function of `weights` only), then per-qubit PauliZ expectations.

Per sample b:
    m_b   = kron_q [cos(a_q/2), sin(a_q/2)]           (real 64-vec, a = x*scale)
    A_b   = [Re(Cc^T) ; Im(Cc^T)] m_b                 (128-vec; Cc folds the
                                                       (-i)^popcount embedding
                                                       phases into U)
    EV_bq = sum_p sgn2[p,q] * A_b[p]^2                (signs of PauliZ)

Device work per core (4096 samples): DMA x shard -> ACT sin/cos -> DVE kron
doubling (batch on partitions) -> PE pair-transposes (two 64-wide groups per
(128,128) transpose) -> packed 64->128 matmul -> ACT square -> sign matmul
(128->6) -> DMA out. Host does only the tiny weights->matrix precompute,
sharding, and the inverse column permutation.
"""
import math
from contextlib import ExitStack

import numpy as np

import concourse.bass as bass
import concourse.mybir as mybir
from concourse.bass_utils import run_bass_kernel_spmd

F32 = mybir.dt.float32
NQ = 6
NL = 6
B = 32768
NCORES = 8
BC = B // NCORES          # 4096 samples per core
NGROUPS = BC // 128       # 32 groups of 128 samples
NSUPER = 4                # super-chunks of 8 groups (1024 samples)


# ---------------------------------------------------------------- host precompute
def _host_matrices(weights):
    """(CcPacked (64,128) f32, SgnZ2 (128,6) f32) from weights (6,6,3)."""
    w = np.asarray(weights, dtype=np.float64)
    phi, theta, omega = w[..., 0], w[..., 1], w[..., 2]
    ct, st = np.cos(0.5 * theta), np.sin(0.5 * theta)
    em = np.exp(-0.5j * (phi + omega))
    ep = np.exp(0.5j * (phi + omega))
    epm = np.exp(0.5j * (phi - omega))
    emp = np.exp(-0.5j * (phi - omega))

    state = np.eye(64, dtype=np.complex128).reshape((64,) + (2,) * NQ)

    def apply_1q(state, U, q):
        ax = q + 1
        s = np.moveaxis(state, ax, -1)
        s = np.einsum('ij,...j->...i', U, s)
        return np.moveaxis(s, -1, ax)

    def cnot(state, c, t):
        ca, ta = c + 1, t + 1
        s0 = np.take(state, 0, axis=ca)
        s1 = np.take(state, 1, axis=ca)
        t_in = ta - 1 if ta > ca else ta
        s1 = np.flip(s1, axis=t_in)
        return np.stack([s0, s1], axis=ca)

    for l in range(NL):
        for q in range(NQ):
            U = np.array([
                [em[l, q] * ct[l, q], -epm[l, q] * st[l, q]],
                [emp[l, q] * st[l, q], ep[l, q] * ct[l, q]],
            ])
            state = apply_1q(state, U, q)
        r = (l % (NQ - 1)) + 1
        for q in range(NQ):
            state = cnot(state, q, (q + r) % NQ)

    stateF = state.reshape(64, 64)            # [in_e, out_o] = U[o, e]
    e = np.arange(64)
    pc = np.array([bin(v).count('1') for v in e])
    phase = (-1j) ** pc                       # (-i)^popcount: RX embedding phases
    Cc = phase[:, None] * stateF              # (64_in, 64_out)

    # device row j has qubit q at bit q; reference index e has qubit 0 as MSB
    bitrev = np.array([int(format(j, '06b')[::-1], 2) for j in range(64)])
    Cdev = Cc[bitrev, :]

    ccpacked = np.concatenate([Cdev.real, Cdev.imag], axis=1)   # (64, 128)

    o = np.arange(64)
    z = np.stack([1.0 - 2.0 * ((o >> (5 - q)) & 1) for q in range(NQ)], axis=1)
    sgn2 = np.concatenate([z, z], axis=0)                        # (128, 6)
    return ccpacked.astype(np.float32), sgn2.astype(np.float32)


def _out_perm():
    """col g in device out (6, BC) holds sample_local perm[g]."""
    perm = np.empty(BC, np.int64)
    for j in range(8):
        h, s = j // 4, j % 4
        c = np.arange(512)
        tp = c // 128
        p_hi = (c % 128) // 32
        pl = c % 32
        perm[j * 512 + c] = 1024 * p_hi + 32 * pl + 8 * s + 2 * tp + h
    return perm


def _lane_sample_index():
    """SL[L, sb]: sample_local for lane L, angle-block sb."""
    L = np.arange(128)
    h, jh, pl = L >> 6, (L >> 5) & 1, L & 31
    sb = np.arange(64)
    s, tp, p_hi = sb >> 4, (sb >> 2) & 3, sb & 3
    return (1024 * p_hi[None, :] + 32 * pl[:, None]
            + 8 * s[None, :] + 2 * tp[None, :] + h[:, None])


_SL = _lane_sample_index()


# ---------------------------------------------------------------- device program
def _build_bass(reps=1):
    """Raw-bass pipeline, ~30 instructions per core per iteration.

    Layout trick: the kron product M is built directly in a 32x32
    block-swizzled layout (lane = (group-parity h, qubit-5 bit, sample
    low bits); host permutes the input accordingly), so a single DVE
    StreamTranspose yields M_T with basis index on partitions - no PE
    transposes at all. Per iteration:

      SP    in-DMA, 2 out-DMAs
      ACT   cos, sin, w(=cos5|sin5 by lane), 2 fat squares, 1 fat EV copy
      DVE   5 fused kron ops (3D broadcast APs), 1 StreamTranspose
      PE    8 packed 64->128 matmuls, 8 sign matmuls (packed into
            PSUM partition bases {0,64})
    """
    nc = bass.Bass()
    # xin cols: [angles 0:384 | pi/2 | wbias | Cc packed twice 386:514
    #            | sgn2 514:520]
    xin = nc.dram_tensor("xin", [128, 520], F32, kind="ExternalInput")
    out = nc.dram_tensor("out", [NQ, BC], F32, kind="ExternalOutput")

    AT, VT, PT = 6, 6, 16

    ctx = ExitStack()
    with ctx:
        sb = lambda nm, shape: ctx.enter_context(nc.sbuf_tensor(nm, shape, F32))
        ps = lambda nm, shape: ctx.enter_context(nc.psum_tensor(nm, shape, F32))
        sem = lambda nm: ctx.enter_context(nc.semaphore(name=nm))

        xt = sb("xt", [128, 520])
        scs = sb("scs", [128, 768])       # cos | sin
        wl = sb("wl", [128, 64])
        k1b = sb("k1b", [128, 256])
        k2b = sb("k2b", [128, 256])
        k3b = sb("k3b", [128, 128])
        m12b = sb("m12b", [128, 1024])
        mswz = sb("mswz", [128, 2048])
        mtall = sb("mtall", [128, 2048])
        ppb = sb("ppb", [128, 4096])
        evst = sb("evst", [128, 2048])
        amall = ps("amall", [128, 2048])
        am2 = ps("am2", [128, 2048])

        Sd, Sa, Sv, Sp, So = (sem("Sd"), sem("Sa"), sem("Sv"), sem("Sp"),
                              sem("So"))

        cc2 = xt.ap()[:, 386:514]
        sg_t = xt.ap()[:, 514:520]

        # 3-free-dim views for the fused kron
        def hsq(q):
            # (p, sb, hf, 1) -> pick angle q, hf = cos/sin half (step 384)
            return (scs.ap()[:, :]
                    .rearrange("p (hf sb q) -> p sb hf q", hf=2, q=NQ)
                    [:, :, :, q:q + 1])

        block = ctx.enter_context(nc.Block())

        @block.sync
        def _(sync):
            for i in range(reps):
                d = sync.dma_start(out=xt.ap()[:, :], in_=xin[:, :])
                if i > 0:
                    d._wait_ge(Sp, PT * i)
                d.then_inc(Sd, 16)
                for b in range(2):
                    dst = (out.rearrange("q (jj bb c) -> q jj bb c",
                                         bb=2, c=512)[:, :, b, :])
                    o = sync.dma_start(
                        out=dst,
                        in_=evst.ap()[64 * b:64 * b + NQ, :]
                            .rearrange("q (jj c) -> q jj c", c=512),
                    )
                    o._wait_ge(Sa, AT * (i + 1)).then_inc(So, 16)
            sync.wait_ge(So, 32 * reps)

        @block.scalar
        def _(scalar):
            sfn = mybir.ActivationFunctionType.Sin
            sqf = mybir.ActivationFunctionType.Square
            for i in range(reps):
                ab, pb = AT * i, PT * i
                c_ = nc.scalar.activation(
                    scs.ap()[:, 0:384], xt.ap()[:, 0:384], sfn,
                    bias=xt.ap()[:, 384:385],
                )
                c_._wait_ge(Sd, 16 * (i + 1)).then_inc(Sa, 1)
                nc.scalar.activation(
                    scs.ap()[:, 384:768], xt.ap()[:, 0:384], sfn,
                ).then_inc(Sa, 1)
                a5 = (xt.ap()[:, 0:384]
                      .rearrange("p (sb q) -> p sb q", q=NQ)[:, :, 5:6])
                w_ = nc.scalar.activation(
                    wl.ap()[:, :].rearrange("p (sb o) -> p sb o", o=1),
                    a5, sfn, bias=xt.ap()[:, 385:386],
                )
                if i > 0:
                    w_._wait_ge(So, 32 * i)
                w_.then_inc(Sa, 1)
                q1 = nc.scalar.activation(ppb.ap()[:, 0:2048],
                                          amall.ap()[:, :], sqf)
                q1._wait_ge(Sp, pb + 4).then_inc(Sa, 1)
                q2 = nc.scalar.activation(ppb.ap()[:, 2048:4096],
                                          am2.ap()[:, :], sqf)
                q2._wait_ge(Sp, pb + 8).then_inc(Sa, 1)
                e_ = nc.scalar.copy(evst.ap()[:, :], amall.ap()[:, :])
                e_._wait_ge(Sp, pb + 16).then_inc(Sa, 1)

        @block.vector
        def _(vector):
            for i in range(reps):
                ab, vb, pb = AT * i, VT * i, PT * i
                # k1 = t0 (x) t1
                o1 = (k1b.ap()[:, :]
                      .rearrange("p (sb b1 b0) -> p sb b1 b0", b1=2, b0=2))
                i0 = hsq(0).squeeze(3).unsqueeze(2).broadcast_to((128, 64, 2, 2))
                i1 = hsq(1).squeeze(3).unsqueeze(3).broadcast_to((128, 64, 2, 2))
                t = nc.vector.tensor_mul(o1, i0, i1)
                t._wait_ge(Sa, ab + 2).then_inc(Sv, 1)
                o2 = (k2b.ap()[:, :]
                      .rearrange("p (sb b3 b2) -> p sb b3 b2", b3=2, b2=2))
                i0 = hsq(2).squeeze(3).unsqueeze(2).broadcast_to((128, 64, 2, 2))
                i1 = hsq(3).squeeze(3).unsqueeze(3).broadcast_to((128, 64, 2, 2))
                t = nc.vector.tensor_mul(o2, i0, i1)
                if i > 0:
                    t._wait_ge(Sp, pb - PT + 8)
                t.then_inc(Sv, 1)
                o3 = (k3b.ap()[:, :]
                      .rearrange("p (sb b4) -> p sb b4", b4=2))
                i0 = hsq(4).squeeze(3)
                i1 = (wl.ap()[:, :].rearrange("p (sb o) -> p sb o", o=1)
                      .broadcast_to((128, 64, 2)))
                t = nc.vector.tensor_mul(o3, i0, i1)
                t._wait_ge(Sa, ab + 3).then_inc(Sv, 1)
                om = (m12b.ap()[:, :]
                      .rearrange("p (sb b32 b10) -> p sb b32 b10", b32=4, b10=4))
                i0 = (k1b.ap()[:, :].rearrange("p (sb w) -> p sb w", w=4)
                      .unsqueeze(2).broadcast_to((128, 64, 4, 4)))
                i1 = (k2b.ap()[:, :].rearrange("p (sb w) -> p sb w", w=4)
                      .unsqueeze(3).broadcast_to((128, 64, 4, 4)))
                t = nc.vector.tensor_mul(om, i0, i1)
                t._wait_ge(Sv, vb + 2).then_inc(Sv, 1)
                oM = (mswz.ap()[:, :]
                      .rearrange("p (sb b4 w) -> p sb b4 w", b4=2, w=16))
                i0 = (m12b.ap()[:, :].rearrange("p (sb w) -> p sb w", w=16)
                      .unsqueeze(2).broadcast_to((128, 64, 2, 16)))
                i1 = (k3b.ap()[:, :].rearrange("p (sb b4) -> p sb b4", b4=2)
                      .unsqueeze(3).broadcast_to((128, 64, 2, 16)))
                t = nc.vector.tensor_mul(oM, i0, i1)
                t._wait_ge(Sv, vb + 4).then_inc(Sv, 1)
                st = nc.vector.transpose(mtall.ap()[:, :], mswz.ap()[:, :])
                st._wait_ge(Sv, vb + 5).then_inc(Sv, 1)

        @block.tensor
        def _(tensor):
            for i in range(reps):
                ab, vb = AT * i, VT * i
                for k in range(8):
                    h, s = divmod(k, 4)
                    dst = [amall, am2][h]
                    mm = nc.tensor.matmul(
                        dst.ap()[:, s * 512:(s + 1) * 512],
                        cc2[64 * h:64 * h + 64, :],
                        mtall.ap()[64 * h:64 * h + 64, s * 512:(s + 1) * 512],
                        start=True, stop=True,
                    )
                    if k == 0:
                        mm._wait_ge(Sv, vb + 6)
                    mm.then_inc(Sp, 1)
                for j in range(8):
                    mm = nc.tensor.matmul(
                        amall.ap()[64 * (j % 2):64 * (j % 2) + NQ,
                                   (j // 2) * 512:(j // 2) * 512 + 512],
                        sg_t, ppb.ap()[:, j * 512:(j + 1) * 512],
                        start=True, stop=True,
                    )
                    if j == 0:
                        mm._wait_ge(Sa, ab + 4)
                    if j == 4:
                        mm._wait_ge(Sa, ab + 5)
                    mm.then_inc(Sp, 1)

    return nc


_CACHE = {}


def _get_nc():
    if "nc" not in _CACHE:
        _CACHE["nc"] = _build_bass()
        _CACHE["perm"] = _out_perm()
    return _CACHE["nc"], _CACHE["perm"]


# ---------------------------------------------------------------- entry point
def _make_in_maps(x, weights, scale):
    x = np.asarray(x, dtype=np.float32)
    ccp, sg2 = _host_matrices(weights)
    hs = 0.5 * float(np.asarray(scale).reshape(-1)[0])
    a = x * hs                                   # (B, 6) half-angles
    L = np.arange(128)
    wbias = np.where(((L >> 5) & 1) == 0, math.pi / 2, 0.0).astype(np.float32)
    in_maps = []
    for k in range(NCORES):
        ak = a[k * BC:(k + 1) * BC]              # (4096, 6)
        xs2 = np.empty((128, 520), np.float32)
        xs2[:, 0:384] = ak[_SL].reshape(128, 384)
        xs2[:, 384] = math.pi / 2
        xs2[:, 385] = wbias
        xs2[0:64, 386:514] = ccp
        xs2[64:128, 386:514] = ccp
        xs2[:, 514:520] = sg2
        in_maps.append({"xin": xs2})
    return in_maps


def kernel(x, weights, scale):
    nc, perm = _get_nc()
    in_maps = _make_in_maps(x, weights, scale)
    res = run_bass_kernel_spmd(nc, in_maps, list(range(NCORES))).results
    ev = np.empty((B, NQ), np.float32)
    for k in range(NCORES):
        ev[k * BC + perm, :] = res[k]["out"].T
    return ev


if __name__ == "__main__":
    rng = np.random.default_rng(0)
    x = rng.standard_normal((B, NQ)).astype(np.float32)
    weights = rng.uniform(0, 2 * np.pi, (NL, NQ, 3)).astype(np.float32)
    scale = np.array([0.1], np.float32)
    ev = kernel(x, weights, scale)
    print("out", ev.shape, ev.dtype, ev[:2])


# revision 20
# speedup vs baseline: 5.1819x; 2.0864x over previous
"""Trainium2 Bass kernel for nn_EnhancedQuantumLayer (6-qubit circuit, B=32768).

Reduction: the circuit is AngleEmbedding (per-sample RX product state) followed
by a batch-independent 64x64 unitary U (StronglyEntanglingLayers + CNOT rings,
function of `weights` only), then per-qubit PauliZ expectations.

Per sample b:
    m_b   = kron_q [cos(a_q/2), sin(a_q/2)]           (real 64-vec, a = x*scale)
    A_b   = [Re(Cc^T) ; Im(Cc^T)] m_b                 (128-vec; Cc folds the
                                                       (-i)^popcount embedding
                                                       phases into U)
    EV_bq = sum_p sgn2[p,q] * A_b[p]^2                (signs of PauliZ)

Device work per core (4096 samples): DMA x shard -> ACT sin/cos -> DVE kron
doubling (batch on partitions) -> PE pair-transposes (two 64-wide groups per
(128,128) transpose) -> packed 64->128 matmul -> ACT square -> sign matmul
(128->6) -> DMA out. Host does only the tiny weights->matrix precompute,
sharding, and the inverse column permutation.
"""
import math
from contextlib import ExitStack

import numpy as np

import concourse.bass as bass
import concourse.mybir as mybir
from concourse.bass_utils import run_bass_kernel_spmd

F32 = mybir.dt.float32
NQ = 6
NL = 6
B = 32768
NCORES = 8
BC = B // NCORES          # 4096 samples per core
NSB = 64                  # angle blocks per lane (s, t', p_hi)


# ---------------------------------------------------------------- host precompute
def _host_matrices(weights):
    """(CcPacked (64,128) f32, SgnZ2 (128,6) f32) from weights (6,6,3)."""
    w = np.asarray(weights, dtype=np.float64)
    phi, theta, omega = w[..., 0], w[..., 1], w[..., 2]
    ct, st = np.cos(0.5 * theta), np.sin(0.5 * theta)
    em = np.exp(-0.5j * (phi + omega))
    ep = np.exp(0.5j * (phi + omega))
    epm = np.exp(0.5j * (phi - omega))
    emp = np.exp(-0.5j * (phi - omega))

    state = np.eye(64, dtype=np.complex128).reshape((64,) + (2,) * NQ)

    def apply_1q(state, U, q):
        ax = q + 1
        s = np.moveaxis(state, ax, -1)
        s = np.einsum('ij,...j->...i', U, s)
        return np.moveaxis(s, -1, ax)

    def cnot(state, c, t):
        ca, ta = c + 1, t + 1
        s0 = np.take(state, 0, axis=ca)
        s1 = np.take(state, 1, axis=ca)
        t_in = ta - 1 if ta > ca else ta
        s1 = np.flip(s1, axis=t_in)
        return np.stack([s0, s1], axis=ca)

    for l in range(NL):
        for q in range(NQ):
            U = np.array([
                [em[l, q] * ct[l, q], -epm[l, q] * st[l, q]],
                [emp[l, q] * st[l, q], ep[l, q] * ct[l, q]],
            ])
            state = apply_1q(state, U, q)
        r = (l % (NQ - 1)) + 1
        for q in range(NQ):
            state = cnot(state, q, (q + r) % NQ)

    stateF = state.reshape(64, 64)            # [in_e, out_o] = U[o, e]
    e = np.arange(64)
    pc = np.array([bin(v).count('1') for v in e])
    phase = (-1j) ** pc                       # (-i)^popcount: RX embedding phases
    Cc = phase[:, None] * stateF              # (64_in, 64_out)

    # device row j has qubit q at bit q; reference index e has qubit 0 as MSB
    bitrev = np.array([int(format(j, '06b')[::-1], 2) for j in range(64)])
    Cdev = Cc[bitrev, :]

    ccpacked = np.concatenate([Cdev.real, Cdev.imag], axis=1)   # (64, 128)

    o = np.arange(64)
    z = np.stack([1.0 - 2.0 * ((o >> (5 - q)) & 1) for q in range(NQ)], axis=1)
    sgn2 = np.concatenate([z, z], axis=0)                        # (128, 6)
    return ccpacked.astype(np.float32), sgn2.astype(np.float32)


def _out_perm():
    """col g in device out (6, BC) holds sample_local perm[g]."""
    perm = np.empty(BC, np.int64)
    for j in range(8):
        h, s = j // 4, j % 4
        c = np.arange(512)
        tp = c // 128
        p_hi = (c % 128) // 32
        pl = c % 32
        perm[j * 512 + c] = 1024 * p_hi + 32 * pl + 8 * s + 2 * tp + h
    return perm


def _lane_sample_index():
    """SL[L, sb]: sample_local for lane L, angle-block sb."""
    L = np.arange(128)
    h, jh, pl = L >> 6, (L >> 5) & 1, L & 31
    sb = np.arange(64)
    s, tp, p_hi = sb >> 4, (sb >> 2) & 3, sb & 3
    return (1024 * p_hi[None, :] + 32 * pl[:, None]
            + 8 * s[None, :] + 2 * tp[None, :] + h[:, None])


_SL = _lane_sample_index()


# ---------------------------------------------------------------- device program
def _build_bass(reps=1):
    """Raw-bass pipeline, ~30 instructions per core per iteration.

    Layout trick: the kron product M is built directly in a 32x32
    block-swizzled layout (lane = (group-parity h, qubit-5 bit, sample
    low bits); host permutes the input accordingly), so a single DVE
    StreamTranspose yields M_T with basis index on partitions - no PE
    transposes at all. Per iteration:

      SP    in-DMA, 2 out-DMAs
      ACT   cos, sin, w(=cos5|sin5 by lane), 2 fat squares, 1 fat EV copy
      DVE   5 fused kron ops (3D broadcast APs), 1 StreamTranspose
      PE    8 packed 64->128 matmuls, 8 sign matmuls (packed into
            PSUM partition bases {0,64})
    """
    nc = bass.Bass()
    # xin cols: [angles 0:384 | pi/2 | wbias | Cc packed twice 386:514
    #            | sgn2 514:520]
    xin = nc.dram_tensor("xin", [128, 520], F32, kind="ExternalInput")
    out = nc.dram_tensor("out", [NQ, BC], F32, kind="ExternalOutput")

    AT, VT, PT = 6, 6, 16

    ctx = ExitStack()
    with ctx:
        sb = lambda nm, shape: ctx.enter_context(nc.sbuf_tensor(nm, shape, F32))
        ps = lambda nm, shape: ctx.enter_context(nc.psum_tensor(nm, shape, F32))
        sem = lambda nm: ctx.enter_context(nc.semaphore(name=nm))

        xt = sb("xt", [128, 520])
        scs = sb("scs", [128, 768])       # cos | sin
        wl = sb("wl", [128, 64])
        k1b = sb("k1b", [128, 256])
        k2b = sb("k2b", [128, 256])
        k3b = sb("k3b", [128, 128])
        m12b = sb("m12b", [128, 1024])
        mswz = sb("mswz", [128, 2048])
        mtall = sb("mtall", [128, 2048])
        ppb = sb("ppb", [128, 4096])
        evst = sb("evst", [128, 2048])
        amall = ps("amall", [128, 2048])
        am2 = ps("am2", [128, 2048])

        Sd, Sa, Sv, Sp, So = (sem("Sd"), sem("Sa"), sem("Sv"), sem("Sp"),
                              sem("So"))

        cc2 = xt.ap()[:, 386:514]
        sg_t = xt.ap()[:, 514:520]

        # 3-free-dim views for the fused kron
        def hsq(q):
            # (p, sb, hf, 1) -> pick angle q, hf = cos/sin half (step 384)
            return (scs.ap()[:, :]
                    .rearrange("p (hf sb q) -> p sb hf q", hf=2, q=NQ)
                    [:, :, :, q:q + 1])

        block = ctx.enter_context(nc.Block())

        @block.sync
        def _(sync):
            for i in range(reps):
                d = sync.dma_start(out=xt.ap()[:, :], in_=xin[:, :])
                if i > 0:
                    d._wait_ge(Sp, PT * i)
                d.then_inc(Sd, 16)
                for b in range(2):
                    dst = (out.rearrange("q (jj bb c) -> q jj bb c",
                                         bb=2, c=512)[:, :, b, :])
                    o = sync.dma_start(
                        out=dst,
                        in_=evst.ap()[64 * b:64 * b + NQ, :]
                            .rearrange("q (jj c) -> q jj c", c=512),
                    )
                    o._wait_ge(Sa, AT * (i + 1)).then_inc(So, 16)
            sync.wait_ge(So, 32 * reps)

        @block.scalar
        def _(scalar):
            sfn = mybir.ActivationFunctionType.Sin
            sqf = mybir.ActivationFunctionType.Square
            for i in range(reps):
                ab, pb = AT * i, PT * i
                c_ = nc.scalar.activation(
                    scs.ap()[:, 0:384], xt.ap()[:, 0:384], sfn,
                    bias=xt.ap()[:, 384:385],
                )
                c_._wait_ge(Sd, 16 * (i + 1)).then_inc(Sa, 1)
                nc.scalar.activation(
                    scs.ap()[:, 384:768], xt.ap()[:, 0:384], sfn,
                ).then_inc(Sa, 1)
                a5 = (xt.ap()[:, 0:384]
                      .rearrange("p (sb q) -> p sb q", q=NQ)[:, :, 5:6])
                w_ = nc.scalar.activation(
                    wl.ap()[:, :].rearrange("p (sb o) -> p sb o", o=1),
                    a5, sfn, bias=xt.ap()[:, 385:386],
                )
                if i > 0:
                    w_._wait_ge(So, 32 * i)
                w_.then_inc(Sa, 1)
                q1 = nc.scalar.activation(ppb.ap()[:, 0:2048],
                                          amall.ap()[:, :], sqf)
                q1._wait_ge(Sp, pb + 4).then_inc(Sa, 1)
                q2 = nc.scalar.activation(ppb.ap()[:, 2048:4096],
                                          am2.ap()[:, :], sqf)
                q2._wait_ge(Sp, pb + 8).then_inc(Sa, 1)
                e_ = nc.scalar.copy(evst.ap()[:, :], amall.ap()[:, :])
                e_._wait_ge(Sp, pb + 16).then_inc(Sa, 1)

        @block.vector
        def _(vector):
            for i in range(reps):
                ab, vb, pb = AT * i, VT * i, PT * i
                # k1 = t0 (x) t1
                o1 = (k1b.ap()[:, :]
                      .rearrange("p (sb b1 b0) -> p sb b1 b0", b1=2, b0=2))
                i0 = hsq(0).squeeze(3).unsqueeze(2).broadcast_to((128, 64, 2, 2))
                i1 = hsq(1).squeeze(3).unsqueeze(3).broadcast_to((128, 64, 2, 2))
                t = nc.vector.tensor_mul(o1, i0, i1)
                t._wait_ge(Sa, ab + 2).then_inc(Sv, 1)
                o2 = (k2b.ap()[:, :]
                      .rearrange("p (sb b3 b2) -> p sb b3 b2", b3=2, b2=2))
                i0 = hsq(2).squeeze(3).unsqueeze(2).broadcast_to((128, 64, 2, 2))
                i1 = hsq(3).squeeze(3).unsqueeze(3).broadcast_to((128, 64, 2, 2))
                t = nc.vector.tensor_mul(o2, i0, i1)
                if i > 0:
                    t._wait_ge(Sp, pb - PT + 8)
                t.then_inc(Sv, 1)
                o3 = (k3b.ap()[:, :]
                      .rearrange("p (sb b4) -> p sb b4", b4=2))
                i0 = hsq(4).squeeze(3)
                i1 = (wl.ap()[:, :].rearrange("p (sb o) -> p sb o", o=1)
                      .broadcast_to((128, 64, 2)))
                t = nc.vector.tensor_mul(o3, i0, i1)
                t._wait_ge(Sa, ab + 3).then_inc(Sv, 1)
                om = (m12b.ap()[:, :]
                      .rearrange("p (sb b32 b10) -> p sb b32 b10", b32=4, b10=4))
                i0 = (k1b.ap()[:, :].rearrange("p (sb w) -> p sb w", w=4)
                      .unsqueeze(2).broadcast_to((128, 64, 4, 4)))
                i1 = (k2b.ap()[:, :].rearrange("p (sb w) -> p sb w", w=4)
                      .unsqueeze(3).broadcast_to((128, 64, 4, 4)))
                t = nc.vector.tensor_mul(om, i0, i1)
                t._wait_ge(Sv, vb + 2).then_inc(Sv, 1)
                oM = (mswz.ap()[:, :]
                      .rearrange("p (sb b4 w) -> p sb b4 w", b4=2, w=16))
                i0 = (m12b.ap()[:, :].rearrange("p (sb w) -> p sb w", w=16)
                      .unsqueeze(2).broadcast_to((128, 64, 2, 16)))
                i1 = (k3b.ap()[:, :].rearrange("p (sb b4) -> p sb b4", b4=2)
                      .unsqueeze(3).broadcast_to((128, 64, 2, 16)))
                t = nc.vector.tensor_mul(oM, i0, i1)
                t._wait_ge(Sv, vb + 4).then_inc(Sv, 1)
                st = nc.vector.transpose(mtall.ap()[:, :], mswz.ap()[:, :])
                st._wait_ge(Sv, vb + 5).then_inc(Sv, 1)

        @block.tensor
        def _(tensor):
            for i in range(reps):
                ab, vb = AT * i, VT * i
                for k in range(8):
                    h, s = divmod(k, 4)
                    dst = [amall, am2][h]
                    mm = nc.tensor.matmul(
                        dst.ap()[:, s * 512:(s + 1) * 512],
                        cc2[64 * h:64 * h + 64, :],
                        mtall.ap()[64 * h:64 * h + 64, s * 512:(s + 1) * 512],
                        start=True, stop=True,
                    )
                    if k == 0:
                        mm._wait_ge(Sv, vb + 6)
                    mm.then_inc(Sp, 1)
                for j in range(8):
                    mm = nc.tensor.matmul(
                        amall.ap()[64 * (j % 2):64 * (j % 2) + NQ,
                                   (j // 2) * 512:(j // 2) * 512 + 512],
                        sg_t, ppb.ap()[:, j * 512:(j + 1) * 512],
                        start=True, stop=True,
                    )
                    if j == 0:
                        mm._wait_ge(Sa, ab + 4)
                    if j == 4:
                        mm._wait_ge(Sa, ab + 5)
                    mm.then_inc(Sp, 1)

    return nc


_CACHE = {}


def _get_nc():
    if "nc" not in _CACHE:
        _CACHE["nc"] = _build_bass()
        _CACHE["perm"] = _out_perm()
    return _CACHE["nc"], _CACHE["perm"]


# ---------------------------------------------------------------- entry point
def _make_in_maps(x, weights, scale):
    x = np.asarray(x, dtype=np.float32)
    ccp, sg2 = _host_matrices(weights)
    hs = 0.5 * float(np.asarray(scale).reshape(-1)[0])
    a = x * hs                                   # (B, 6) half-angles
    L = np.arange(128)
    wbias = np.where(((L >> 5) & 1) == 0, math.pi / 2, 0.0).astype(np.float32)
    in_maps = []
    for k in range(NCORES):
        ak = a[k * BC:(k + 1) * BC]              # (4096, 6)
        xs2 = np.empty((128, 520), np.float32)
        xs2[:, 0:384] = ak[_SL].reshape(128, 384)
        xs2[:, 384] = math.pi / 2
        xs2[:, 385] = wbias
        xs2[0:64, 386:514] = ccp
        xs2[64:128, 386:514] = ccp
        xs2[:, 514:520] = sg2
        in_maps.append({"xin": xs2})
    return in_maps


def kernel(x, weights, scale):
    nc, perm = _get_nc()
    in_maps = _make_in_maps(x, weights, scale)
    res = run_bass_kernel_spmd(nc, in_maps, list(range(NCORES))).results
    ev = np.empty((B, NQ), np.float32)
    for k in range(NCORES):
        ev[k * BC + perm, :] = res[k]["out"].T
    return ev


if __name__ == "__main__":
    rng = np.random.default_rng(0)
    x = rng.standard_normal((B, NQ)).astype(np.float32)
    weights = rng.uniform(0, 2 * np.pi, (NL, NQ, 3)).astype(np.float32)
    scale = np.array([0.1], np.float32)
    ev = kernel(x, weights, scale)
    print("out", ev.shape, ev.dtype, ev[:2])


# revision 22
# speedup vs baseline: 7.6517x; 1.4766x over previous
"""Trainium2 Bass kernel for nn_EnhancedQuantumLayer (6-qubit circuit, B=32768).

Reduction: the circuit is AngleEmbedding (per-sample RX product state) followed
by a batch-independent 64x64 unitary U (StronglyEntanglingLayers + CNOT rings,
function of `weights` only), then per-qubit PauliZ expectations.

Per sample b:
    m_b   = kron_q [cos(a_q/2), sin(a_q/2)]           (real 64-vec, a = x*scale)
    A_b   = [Re(Cc^T) ; Im(Cc^T)] m_b                 (128-vec; Cc folds the
                                                       (-i)^popcount embedding
                                                       phases into U)
    EV_bq = sum_p sgn2[p,q] * A_b[p]^2                (signs of PauliZ)

Device work per core (4096 samples): DMA x shard -> ACT sin/cos -> DVE kron
doubling (batch on partitions) -> PE pair-transposes (two 64-wide groups per
(128,128) transpose) -> packed 64->128 matmul -> ACT square -> sign matmul
(128->6) -> DMA out. Host does only the tiny weights->matrix precompute,
sharding, and the inverse column permutation.
"""
import math
from contextlib import ExitStack

import numpy as np

import concourse.bass as bass
import concourse.mybir as mybir
from concourse.bass_utils import run_bass_kernel_spmd

F32 = mybir.dt.float32
NQ = 6
NL = 6
B = 32768
NCORES = 8
BC = B // NCORES          # 4096 samples per core
NSB = 64                  # angle blocks per lane (s, t', p_hi)


# ---------------------------------------------------------------- host precompute
def _host_matrices(weights):
    """(CcPacked (64,128) f32, SgnZ2 (128,6) f32) from weights (6,6,3)."""
    w = np.asarray(weights, dtype=np.float64)
    phi, theta, omega = w[..., 0], w[..., 1], w[..., 2]
    ct, st = np.cos(0.5 * theta), np.sin(0.5 * theta)
    em = np.exp(-0.5j * (phi + omega))
    ep = np.exp(0.5j * (phi + omega))
    epm = np.exp(0.5j * (phi - omega))
    emp = np.exp(-0.5j * (phi - omega))

    state = np.eye(64, dtype=np.complex128).reshape((64,) + (2,) * NQ)

    def apply_1q(state, U, q):
        ax = q + 1
        s = np.moveaxis(state, ax, -1)
        s = np.einsum('ij,...j->...i', U, s)
        return np.moveaxis(s, -1, ax)

    def cnot(state, c, t):
        ca, ta = c + 1, t + 1
        s0 = np.take(state, 0, axis=ca)
        s1 = np.take(state, 1, axis=ca)
        t_in = ta - 1 if ta > ca else ta
        s1 = np.flip(s1, axis=t_in)
        return np.stack([s0, s1], axis=ca)

    for l in range(NL):
        for q in range(NQ):
            U = np.array([
                [em[l, q] * ct[l, q], -epm[l, q] * st[l, q]],
                [emp[l, q] * st[l, q], ep[l, q] * ct[l, q]],
            ])
            state = apply_1q(state, U, q)
        r = (l % (NQ - 1)) + 1
        for q in range(NQ):
            state = cnot(state, q, (q + r) % NQ)

    stateF = state.reshape(64, 64)            # [in_e, out_o] = U[o, e]
    e = np.arange(64)
    pc = np.array([bin(v).count('1') for v in e])
    phase = (-1j) ** pc                       # (-i)^popcount: RX embedding phases
    Cc = phase[:, None] * stateF              # (64_in, 64_out)

    # device row j has qubit q at bit q; reference index e has qubit 0 as MSB
    bitrev = np.array([int(format(j, '06b')[::-1], 2) for j in range(64)])
    Cdev = Cc[bitrev, :]

    ccpacked = np.concatenate([Cdev.real, Cdev.imag], axis=1)   # (64, 128)

    o = np.arange(64)
    z = np.stack([1.0 - 2.0 * ((o >> (5 - q)) & 1) for q in range(NQ)], axis=1)
    sgn2 = np.concatenate([z, z], axis=0)                        # (128, 6)
    return ccpacked.astype(np.float32), sgn2.astype(np.float32)


def _out_perm():
    """col g in device out (6, BC) holds sample_local perm[g]."""
    perm = np.empty(BC, np.int64)
    for j in range(8):
        h, s = j // 4, j % 4
        c = np.arange(512)
        tp = c // 128
        p_hi = (c % 128) // 32
        pl = c % 32
        perm[j * 512 + c] = 1024 * p_hi + 32 * pl + 8 * s + 2 * tp + h
    return perm


def _lane_sample_index():
    """SL[L, sb]: sample_local for lane L, angle-block sb."""
    L = np.arange(128)
    h, jh, pl = L >> 6, (L >> 5) & 1, L & 31
    sb = np.arange(64)
    s, tp, p_hi = sb >> 4, (sb >> 2) & 3, sb & 3
    return (1024 * p_hi[None, :] + 32 * pl[:, None]
            + 8 * s[None, :] + 2 * tp[None, :] + h[:, None])


_SL = _lane_sample_index()


# ---------------------------------------------------------------- device program
def _build_bass(reps=1):
    """Raw-bass pipeline, ~30 instructions per core per iteration.

    Layout trick: the kron product M is built directly in a 32x32
    block-swizzled layout (lane = (group-parity h, qubit-5 bit, sample
    low bits); host permutes the input accordingly), so a single DVE
    StreamTranspose yields M_T with basis index on partitions - no PE
    transposes at all. Per iteration:

      SP    in-DMA, 2 out-DMAs
      ACT   cos, sin, w(=cos5|sin5 by lane), 2 fat squares, 1 fat EV copy
      DVE   5 fused kron ops (3D broadcast APs), 1 StreamTranspose
      PE    8 packed 64->128 matmuls, 8 sign matmuls (packed into
            PSUM partition bases {0,64})
    """
    nc = bass.Bass()
    # xin cols: [angles 0:384 | pi/2 | wbias | Cc packed twice 386:514
    #            | sgn2 514:520]
    xin = nc.dram_tensor("xin", [128, 520], F32, kind="ExternalInput")
    out = nc.dram_tensor("out", [NQ, BC], F32, kind="ExternalOutput")

    AT, VT, PT = 6, 6, 16

    ctx = ExitStack()
    with ctx:
        sb = lambda nm, shape: ctx.enter_context(nc.sbuf_tensor(nm, shape, F32))
        ps = lambda nm, shape: ctx.enter_context(nc.psum_tensor(nm, shape, F32))
        sem = lambda nm: ctx.enter_context(nc.semaphore(name=nm))

        xt = sb("xt", [128, 520])
        scs = sb("scs", [128, 768])       # cos | sin
        wl = sb("wl", [128, 64])
        k1b = sb("k1b", [128, 256])
        k2b = sb("k2b", [128, 256])
        k3b = sb("k3b", [128, 128])
        m12b = sb("m12b", [128, 1024])
        mswz = sb("mswz", [128, 2048])
        mtall = sb("mtall", [128, 2048])
        ppb = sb("ppb", [128, 4096])
        evst = sb("evst", [128, 2048])
        amall = ps("amall", [128, 2048])
        am2 = ps("am2", [128, 2048])

        Sd, Sa, Sv, Sp, So = (sem("Sd"), sem("Sa"), sem("Sv"), sem("Sp"),
                              sem("So"))

        cc2 = xt.ap()[:, 386:514]
        sg_t = xt.ap()[:, 514:520]

        # 3-free-dim views for the fused kron
        def hsq(q):
            # (p, sb, hf, 1) -> pick angle q, hf = cos/sin half (step 384)
            return (scs.ap()[:, :]
                    .rearrange("p (hf sb q) -> p sb hf q", hf=2, q=NQ)
                    [:, :, :, q:q + 1])

        block = ctx.enter_context(nc.Block())

        @block.sync
        def _(sync):
            for i in range(reps):
                d = sync.dma_start(out=xt.ap()[:, :], in_=xin[:, :])
                if i > 0:
                    d._wait_ge(Sp, PT * i)
                d.then_inc(Sd, 16)
                for b in range(2):
                    dst = (out.rearrange("q (jj bb c) -> q jj bb c",
                                         bb=2, c=512)[:, :, b, :])
                    o = sync.dma_start(
                        out=dst,
                        in_=evst.ap()[64 * b:64 * b + NQ, :]
                            .rearrange("q (jj c) -> q jj c", c=512),
                    )
                    o._wait_ge(Sa, AT * (i + 1)).then_inc(So, 16)
            sync.wait_ge(So, 32 * reps)

        @block.scalar
        def _(scalar):
            sfn = mybir.ActivationFunctionType.Sin
            sqf = mybir.ActivationFunctionType.Square
            for i in range(reps):
                ab, pb = AT * i, PT * i
                c_ = nc.scalar.activation(
                    scs.ap()[:, 0:384], xt.ap()[:, 0:384], sfn,
                    bias=xt.ap()[:, 384:385],
                )
                c_._wait_ge(Sd, 16 * (i + 1))
                nc.scalar.activation(
                    scs.ap()[:, 384:768], xt.ap()[:, 0:384], sfn,
                ).then_inc(Sa, 2)
                a5 = (xt.ap()[:, 0:384]
                      .rearrange("p (sb q) -> p sb q", q=NQ)[:, :, 5:6])
                w_ = nc.scalar.activation(
                    wl.ap()[:, :].rearrange("p (sb o) -> p sb o", o=1),
                    a5, sfn, bias=xt.ap()[:, 385:386],
                )
                if i > 0:
                    w_._wait_ge(So, 32 * i)
                w_.then_inc(Sa, 1)
                q1 = nc.scalar.activation(ppb.ap()[:, 0:2048],
                                          amall.ap()[:, :], sqf)
                q1._wait_ge(Sp, pb + 4).then_inc(Sa, 1)
                q2 = nc.scalar.activation(ppb.ap()[:, 2048:4096],
                                          am2.ap()[:, :], sqf)
                q2._wait_ge(Sp, pb + 8).then_inc(Sa, 1)
                e_ = nc.scalar.copy(evst.ap()[:, :], amall.ap()[:, :])
                e_._wait_ge(Sp, pb + 16).then_inc(Sa, 1)

        @block.vector
        def _(vector):
            for i in range(reps):
                ab, vb, pb = AT * i, VT * i, PT * i
                # k1 = t0 (x) t1
                o1 = (k1b.ap()[:, :]
                      .rearrange("p (sb b1 b0) -> p sb b1 b0", b1=2, b0=2))
                i0 = hsq(0).squeeze(3).unsqueeze(2).broadcast_to((128, 64, 2, 2))
                i1 = hsq(1).squeeze(3).unsqueeze(3).broadcast_to((128, 64, 2, 2))
                t = nc.vector.tensor_mul(o1, i0, i1)
                t._wait_ge(Sa, ab + 2).then_inc(Sv, 1)
                o2 = (k2b.ap()[:, :]
                      .rearrange("p (sb b3 b2) -> p sb b3 b2", b3=2, b2=2))
                i0 = hsq(2).squeeze(3).unsqueeze(2).broadcast_to((128, 64, 2, 2))
                i1 = hsq(3).squeeze(3).unsqueeze(3).broadcast_to((128, 64, 2, 2))
                t = nc.vector.tensor_mul(o2, i0, i1)
                if i > 0:
                    t._wait_ge(Sp, pb - PT + 8)
                t.then_inc(Sv, 1)
                o3 = (k3b.ap()[:, :]
                      .rearrange("p (sb b4) -> p sb b4", b4=2))
                i0 = hsq(4).squeeze(3)
                i1 = (wl.ap()[:, :].rearrange("p (sb o) -> p sb o", o=1)
                      .broadcast_to((128, 64, 2)))
                t = nc.vector.tensor_mul(o3, i0, i1)
                t._wait_ge(Sa, ab + 3).then_inc(Sv, 1)
                om = (m12b.ap()[:, :]
                      .rearrange("p (sb b32 b10) -> p sb b32 b10", b32=4, b10=4))
                i0 = (k1b.ap()[:, :].rearrange("p (sb w) -> p sb w", w=4)
                      .unsqueeze(2).broadcast_to((128, 64, 4, 4)))
                i1 = (k2b.ap()[:, :].rearrange("p (sb w) -> p sb w", w=4)
                      .unsqueeze(3).broadcast_to((128, 64, 4, 4)))
                t = nc.vector.tensor_mul(om, i0, i1)
                t._wait_ge(Sv, vb + 2).then_inc(Sv, 1)
                oM = (mswz.ap()[:, :]
                      .rearrange("p (sb b4 w) -> p sb b4 w", b4=2, w=16))
                i0 = (m12b.ap()[:, :].rearrange("p (sb w) -> p sb w", w=16)
                      .unsqueeze(2).broadcast_to((128, 64, 2, 16)))
                i1 = (k3b.ap()[:, :].rearrange("p (sb b4) -> p sb b4", b4=2)
                      .unsqueeze(3).broadcast_to((128, 64, 2, 16)))
                t = nc.vector.tensor_mul(oM, i0, i1)
                t._wait_ge(Sv, vb + 4).then_inc(Sv, 1)
                st = nc.vector.transpose(mtall.ap()[:, :], mswz.ap()[:, :])
                st._wait_ge(Sv, vb + 5).then_inc(Sv, 1)

        @block.tensor
        def _(tensor):
            for i in range(reps):
                ab, vb = AT * i, VT * i
                for k in range(8):
                    h, s = divmod(k, 4)
                    dst = [amall, am2][h]
                    mm = nc.tensor.matmul(
                        dst.ap()[:, s * 512:(s + 1) * 512],
                        cc2[64 * h:64 * h + 64, :],
                        mtall.ap()[64 * h:64 * h + 64, s * 512:(s + 1) * 512],
                        start=True, stop=True,
                    )
                    if k == 0:
                        mm._wait_ge(Sv, vb + 6)
                    if k in (3, 7):
                        mm.then_inc(Sp, 4)
                for j in range(8):
                    mm = nc.tensor.matmul(
                        amall.ap()[64 * (j % 2):64 * (j % 2) + NQ,
                                   (j // 2) * 512:(j // 2) * 512 + 512],
                        sg_t, ppb.ap()[:, j * 512:(j + 1) * 512],
                        start=True, stop=True,
                    )
                    if j == 0:
                        mm._wait_ge(Sa, ab + 4)
                    if j == 4:
                        mm._wait_ge(Sa, ab + 5)
                    if j == 7:
                        mm.then_inc(Sp, 8)

    return nc


_CACHE = {}


def _get_nc():
    if "nc" not in _CACHE:
        _CACHE["nc"] = _build_bass()
        _CACHE["perm"] = _out_perm()
    return _CACHE["nc"], _CACHE["perm"]


# ---------------------------------------------------------------- entry point
def _make_in_maps(x, weights, scale):
    x = np.asarray(x, dtype=np.float32)
    ccp, sg2 = _host_matrices(weights)
    hs = 0.5 * float(np.asarray(scale).reshape(-1)[0])
    a = x * hs                                   # (B, 6) half-angles
    L = np.arange(128)
    wbias = np.where(((L >> 5) & 1) == 0, math.pi / 2, 0.0).astype(np.float32)
    in_maps = []
    for k in range(NCORES):
        ak = a[k * BC:(k + 1) * BC]              # (4096, 6)
        xs2 = np.empty((128, 520), np.float32)
        xs2[:, 0:384] = ak[_SL].reshape(128, 384)
        xs2[:, 384] = math.pi / 2
        xs2[:, 385] = wbias
        xs2[0:64, 386:514] = ccp
        xs2[64:128, 386:514] = ccp
        xs2[:, 514:520] = sg2
        in_maps.append({"xin": xs2})
    return in_maps


def kernel(x, weights, scale):
    nc, perm = _get_nc()
    in_maps = _make_in_maps(x, weights, scale)
    res = run_bass_kernel_spmd(nc, in_maps, list(range(NCORES))).results
    ev = np.empty((B, NQ), np.float32)
    for k in range(NCORES):
        ev[k * BC + perm, :] = res[k]["out"].T
    return ev


if __name__ == "__main__":
    rng = np.random.default_rng(0)
    x = rng.standard_normal((B, NQ)).astype(np.float32)
    weights = rng.uniform(0, 2 * np.pi, (NL, NQ, 3)).astype(np.float32)
    scale = np.array([0.1], np.float32)
    ev = kernel(x, weights, scale)
    print("out", ev.shape, ev.dtype, ev[:2])
